# revision 28
# baseline (speedup 1.0000x reference)
"""Causal self-attention (B=4, T=2048, C=1024, H=16) on 8 TRN2 NeuronCores.

Sharding: core c = (b, hg) with b = c//2 batch index, hg = c%2 head-group
(8 heads each).  Each core computes its batch element's attention for its 8
heads plus the partial c_proj (W_proj column-shard); the host sums the two
head-group partials per batch element.

v3 (vs v2):
  - inputs (xT, WqkT, WvT) shipped in bf16: halves input DMA; stage-1
    matmuls run bf16 (same 1 cyc/row as f32r on PE).
  - S/O interleaved emission per (tqb, pc): the stage-3 matmuls for s-tile
    st-1 are emitted between the stage-2 matmuls of st, so the PE never
    head-of-line blocks on psum-bank recycling or on exp.
  - boundary s-tiles: stage-2 matmuls compute only the causally valid
    columns; exp is one merged instruction covering both heads (strided
    AP); the dead-strip memset and the triangle mask are one instruction
    each covering both heads.
  - normalize: one s2 psum tile holds both heads' reciprocal broadcasts
    (two 512-col matmuls); the yT muls read psO and psR directly (two
    PSUM operands); both heads' ytb shifted to partitions 64:127 with one
    DMA per tqb.

`reps` > 1 repeats the whole body inside one NEFF (for wall-clock timing by
differencing, since per-dispatch overhead through axon is ~70-90 ms).
"""
import numpy as np
import ml_dtypes

import concourse.bacc as bacc
import concourse.mybir as mybir
import concourse.tile as tile
from concourse.bass import broadcast_tensor_aps
from concourse.bass_utils import run_bass_kernel_spmd

F32 = mybir.dt.float32
F32R = mybir.dt.float32r
BF16 = mybir.dt.bfloat16

B, C, NH, HD = 4, 1024, 16, 64
HPC = 8              # heads per core
JV = HPC * HD        # 512: v-feature cols per core
KC = C // 128        # 8 contraction chunks
SCALE = 1.0 / 8.0    # 1/sqrt(HD)

# Schraudolph fast-exp in bf16 via DVE (offloads the ACT engine, which
# paces the attention loop on HW): bf16 bitpattern of exp(s/8) is
# approximately round(s * (128*log2(e)/8) + SEXP_C2); computed as one
# tensor_scalar (mult+add) with an int16-bitcast output AP.  Max rel
# error ~3.3% on individual weights; cancels to ~1e-3 after softmax
# normalization (numerator and denominator use the same approx values).
SEXP_C1 = 184.6617 / 8.0
SEXP_C2 = 16250.75


def emit_body(nc, tc, dram, T):
    TT = T // 128
    TQB = T // 512
    xT, wqkT, wvT, wpTb, tri, yout = (
        dram["xT"], dram["wqkT"], dram["wvT"], dram["wpTb"],
        dram["tri"], dram["yout"])

    with tc.tile_pool(name="persist", bufs=1) as pers:
        qkT_sb = pers.tile([128, 8, T], F32R)          # [j-part, jc, t]
        vext_sb = pers.tile([128, TT, HPC, 65], BF16)  # [s-part, st, h, d|1]
        tri2_sb = pers.tile([128, 2, 128], BF16)       # tri[p,c]=1 iff c>=p,
        selt = pers.tile([65, 64], F32R)               # ones row at p=64
        nc.sync.dma_start(tri2_sb[:, 0, :], tri[:])    # duplicated per head
        nc.sync.dma_start(tri2_sb[:, 1, :], tri[:])
        nc.sync.dma_start(selt[64:65, :], dram["ones64"][:])

        with tc.tile_pool(name="mmx", bufs=2, space="PSUM") as ps512:
            # ---------------- stage 1 ----------------
            with tc.tile_pool(name="stage1", bufs=1) as s1p:
                xT_sb = s1p.tile([128, KC, T], BF16)
                wqk_sb = s1p.tile([128, KC, 1024], BF16)
                wv_sb = s1p.tile([128, KC, JV], BF16)
                xT3 = xT.rearrange("(kc p) t -> p kc t", p=128)
                wqk3 = wqkT.rearrange("(kc p) j -> p kc j", p=128)
                wv3 = wvT.rearrange("(kc p) j -> p kc j", p=128)
                # DMA priority order: the first qk psum chains need wqk[kc]
                # plus the nb=0 slice of xT[kc]; ship those first so the PE
                # starts ~2 us in, then stream the rest of xT, then wv.
                for kc in range(KC):
                    nc.sync.dma_start(wqk_sb[:, kc, :], wqk3[:, kc, :])
                    nc.sync.dma_start(xT_sb[:, kc, 0:512], xT3[:, kc, 0:512])
                for kc in range(KC):
                    nc.sync.dma_start(xT_sb[:, kc, 512:T], xT3[:, kc, 512:T])
                for kc in range(KC):
                    nc.sync.dma_start(wv_sb[:, kc, :], wv3[:, kc, :])

                # qkT = WqkT.T-contract(xT): out chunk jc over t blocks
                for nb in range(T // 512):
                    for jc in range(8):
                        ps = ps512.tile([128, 512], F32, tag="ps512")
                        for kc in range(KC):
                            nc.tensor.matmul(
                                ps[:],
                                wqk_sb[:, kc, jc * 128:(jc + 1) * 128],
                                xT_sb[:, kc, nb * 512:(nb + 1) * 512],
                                start=(kc == 0), stop=(kc == KC - 1))
                        nc.vector.tensor_copy(
                            qkT_sb[:, jc, nb * 512:(nb + 1) * 512], ps[:])
                # V = xT.T-contract(WvT): out t-chunk tt, 512 v-cols
                for tt in range(TT):
                    ps = ps512.tile([128, 512], F32, tag="ps512")
                    for kc in range(KC):
                        nc.tensor.matmul(
                            ps[:],
                            xT_sb[:, kc, tt * 128:(tt + 1) * 128],
                            wv_sb[:, kc, :],
                            start=(kc == 0), stop=(kc == KC - 1))
                    nc.vector.tensor_copy(
                        vext_sb[:, tt, :, 0:64],
                        ps[:].rearrange("p (h d) -> p h d", h=HPC))
                    nc.vector.memset(vext_sb[:, tt, :, 64:65], 1.0)

            # ---------------- attention + proj ----------------
            with tc.tile_pool(name="s2ps", bufs=2, space="PSUM") as s2ps, \
                 tc.tile_pool(name="ps3p", bufs=2, space="PSUM") as ps3p, \
                 tc.tile_pool(name="wp", bufs=1) as wpp, \
                 tc.tile_pool(name="pexp", bufs=2) as ppool, \
                 tc.tile_pool(name="ytpool", bufs=2) as ytpool, \
                 tc.tile_pool(name="ybpool", bufs=2) as ybpool, \
                 tc.tile_pool(name="rbpool", bufs=4) as rbpool, \
                 tc.tile_pool(name="outp", bufs=3) as outp:
                wp_sb = wpp.tile([128, 4, C], BF16)
                wp3 = wpTb.rearrange("(jc p) co -> p jc co", p=128)
                for jc in range(4):
                    nc.sync.dma_start(wp_sb[:, jc, :], wp3[:, jc, :])

                def emit_s4_chunk(yT_s4, tqb_s4, k):
                    """Stage-4 chunk k (0..7) of tq block tqb_s4: one psum
                    group = 128 t-rows x 512 out-cols."""
                    sub, nb2 = k // 2, k % 2
                    ps4 = ps512.tile([128, 512], F32, tag="ps512")
                    for jc in range(4):
                        nc.tensor.matmul(
                            ps4[:],
                            yT_s4[:, jc, sub * 128:(sub + 1) * 128],
                            wp_sb[:, jc, nb2 * 512:(nb2 + 1) * 512],
                            start=(jc == 0), stop=(jc == 3))
                    ot = outp.tile([128, 512], F32, tag="ot")
                    nc.vector.tensor_copy(ot[:], ps4[:])
                    t0 = (tqb_s4 * 4 + sub) * 128
                    nc.sync.dma_start(
                        yout[t0:t0 + 128, nb2 * 512:(nb2 + 1) * 512],
                        ot[:])

                yT_prev = None
                for tqb in range(TQB):
                    nst = 4 * (tqb + 1)     # causal: s-tiles 0..nst-1
                    yT_t = ytpool.tile([128, 4, 512], BF16, tag="yt")
                    for pc in range(4):
                        pab = ppool.tile([128, TT, 1024], BF16, tag="pab")
                        qs = 2 * pc         # chunk with [Qa|Qb]
                        ks = 2 * pc + 1     # chunk with [Ka|Kb]
                        tqs = slice(tqb * 512, (tqb + 1) * 512)
                        psOa = ps3p.tile([128, 512], F32, tag="s3")
                        psOb = ps3p.tile([128, 512], F32, tag="s3")
                        psOs = (psOa, psOb)

                        def emit_o(st, start, stop):
                            for hoff in (0, 1):
                                nc.tensor.matmul(
                                    psOs[hoff][0:65, :],
                                    vext_sb[:, st, 2 * pc + hoff, :],
                                    pab[:, st, hoff * 512:(hoff + 1) * 512],
                                    start=start, stop=stop)

                        for st in range(nst):
                            ss = slice(st * 128, (st + 1) * 128)
                            psAB = s2ps.tile([128, 1024], F32, tag="s2")
                            q = st - 4 * tqb
                            c0 = max(q, 0) * 128   # first valid col
                            tqv = slice(tqb * 512 + c0, (tqb + 1) * 512)
                            nc.tensor.matmul(
                                psAB[:, c0:512], qkT_sb[0:64, ks, ss],
                                qkT_sb[0:64, qs, tqv],
                                start=True, stop=True, tile_position=(0, 0))
                            nc.tensor.matmul(
                                psAB[:, 512 + c0:1024], qkT_sb[64:128, ks, ss],
                                qkT_sb[64:128, qs, tqv],
                                start=True, stop=True, tile_position=(64, 0))
                            pab2 = pab[:, st, :].rearrange(
                                "p (h c) -> p h c", h=2)
                            psAB2 = psAB.rearrange("p (h c) -> p h c", h=2)
                            if q < 0:       # fully-valid s-tile
                                if st % 4 == 1:   # DVE fast-exp offload
                                    with nc.allow_low_precision(
                                            reason="schraudolph bf16 exp"):
                                        nc.vector.tensor_scalar(
                                            pab[:, st, :].bitcast(
                                                mybir.dt.int16),
                                            psAB[:], SEXP_C1, SEXP_C2,
                                            mybir.AluOpType.mult,
                                            mybir.AluOpType.add)
                                else:
                                    nc.scalar.activation(
                                        pab[:, st, :], psAB[:],
                                        mybir.ActivationFunctionType.Exp,
                                        scale=SCALE)
                            else:           # boundary s-tile: causal edge
                                if q > 0:
                                    nc.gpsimd.memset(pab2[:, :, 0:c0], 0.0)
                                nc.scalar.activation(
                                    pab2[:, :, c0:512], psAB2[:, :, c0:512],
                                    mybir.ActivationFunctionType.Exp,
                                    scale=SCALE)
                                mpa = pab2[:, :, c0:c0 + 128]
                                nc.gpsimd.tensor_mul(mpa, mpa, tri2_sb[:])
                            if st >= 2:
                                emit_o(st - 2, start=(st == 2), stop=False)
                        emit_o(nst - 2, start=(nst == 2), stop=False)
                        emit_o(nst - 1, start=False, stop=True)

                        # normalize: yT_h = OT_h * (1/denom_h); reciprocal of
                        # the denom row stays on partition 64 (lane-aligned),
                        # then a K=1 ones-row matmul broadcasts it across 64
                        # output partitions.
                        rcp2 = rbpool.tile([65, 2, 512], F32R, tag="rcp")
                        with nc.allow_low_precision(
                                reason="f32r is bit-identical to f32; "
                                       "needed for 1-cyc/row PE broadcast"):
                            nc.vector.reciprocal(
                                rcp2[64:65, 0, :], psOa[64:65, :])
                            nc.vector.reciprocal(
                                rcp2[64:65, 1, :], psOb[64:65, :])
                        psRa = ps512.tile([128, 512], F32, tag="ps512")
                        psRb = ps512.tile([128, 512], F32, tag="ps512")
                        nc.tensor.matmul(
                            psRa[0:64, :], selt[64:65, :],
                            rcp2[64:65, 0, :], start=True, stop=True)
                        nc.tensor.matmul(
                            psRb[0:64, :], selt[64:65, :],
                            rcp2[64:65, 1, :], start=True, stop=True)
                        rba = rbpool.tile([64, 512], F32, tag="rba")
                        nc.vector.tensor_copy(rba[:], psRa[0:64, :])
                        rbb = rbpool.tile([64, 512], F32, tag="rbb")
                        nc.vector.tensor_copy(rbb[:], psRb[0:64, :])
                        ytb = ybpool.tile([64, 512], BF16, tag="ytb")
                        with nc.allow_low_precision(
                                reason="attn output feeds bf16 c_proj"):
                            nc.vector.tensor_mul(
                                yT_t[0:64, pc, :], psOa[0:64, :], rba[:])
                            nc.vector.tensor_mul(
                                ytb[:], psOb[0:64, :], rbb[:])
                        nc.sync.dma_start(yT_t[64:128, pc, :], ytb[:])
                        # stage 4 of the previous tq block: PE filler that
                        # has no ACT/DVE dependency, absorbing exp-lag and
                        # normalize-latency stalls.
                        if yT_prev is not None:
                            emit_s4_chunk(yT_prev, tqb - 1, 2 * pc)
                            emit_s4_chunk(yT_prev, tqb - 1, 2 * pc + 1)
                    yT_prev = yT_t
                # stage 4 of the last tq block
                for k in range(8):
                    emit_s4_chunk(yT_prev, TQB - 1, k)


def build_nc(T=2048, reps=1):
    nc = bacc.Bacc()
    dram = dict(
        xT=nc.dram_tensor("xT", [C, T], BF16, kind="ExternalInput"),
        wqkT=nc.dram_tensor("wqkT", [C, 1024], BF16, kind="ExternalInput"),
        wvT=nc.dram_tensor("wvT", [C, JV], BF16, kind="ExternalInput"),
        wpTb=nc.dram_tensor("wpTb", [JV, C], BF16, kind="ExternalInput"),
        tri=nc.dram_tensor("tri", [128, 128], BF16, kind="ExternalInput"),
        ones64=nc.dram_tensor("ones64", [1, 64], F32R, kind="ExternalInput"),
        yout=nc.dram_tensor("yout", [T, C], F32, kind="ExternalOutput"),
    )
    with tile.TileContext(nc) as tc:
        for _ in range(reps):
            emit_body(nc, tc, dram, T)
    nc.compile()
    return nc


def shard_inputs(x, W_attn, W_proj, T):
    """Full inputs -> list of 8 per-core in_maps."""
    x = np.asarray(x, dtype=np.float32)
    W_attn = np.asarray(W_attn, dtype=np.float32)
    W_proj = np.asarray(W_proj, dtype=np.float32)

    p = np.arange(128)[:, None]
    c = np.arange(128)[None, :]
    tri = (c >= p).astype(ml_dtypes.bfloat16)

    in_maps = []
    for core in range(8):
        b, hg = core // 2, core % 2
        heads = [hg * HPC + i for i in range(HPC)]
        cols = []
        for pc in range(4):
            ha, hb = heads[2 * pc], heads[2 * pc + 1]
            cols += list(range(ha * 192, ha * 192 + 64))        # Q_a
            cols += list(range(hb * 192, hb * 192 + 64))        # Q_b
            cols += list(range(ha * 192 + 64, ha * 192 + 128))  # K_a
            cols += list(range(hb * 192 + 64, hb * 192 + 128))  # K_b
        vrows = [h * 192 + 128 + d for h in heads for d in range(64)]
        in_maps.append(dict(
            xT=np.ascontiguousarray(x[b, :T].T).astype(ml_dtypes.bfloat16),
            wqkT=np.ascontiguousarray(
                W_attn[cols].T).astype(ml_dtypes.bfloat16),
            wvT=np.ascontiguousarray(
                W_attn[vrows].T).astype(ml_dtypes.bfloat16),
            tri=tri,
            ones64=np.ones((1, 64), dtype=np.float32),
            wpTb=np.ascontiguousarray(
                W_proj[:, hg * JV:(hg + 1) * JV].T).astype(ml_dtypes.bfloat16),
        ))
    return in_maps


def gather_outputs(results, T):
    out = np.empty((B, T, C), dtype=np.float32)
    for b in range(B):
        out[b] = results[2 * b]["yout"] + results[2 * b + 1]["yout"]
    return out


_NC_CACHE = {}


def run(x, W_attn, W_proj, T=2048, trace=False):
    if T not in _NC_CACHE:
        _NC_CACHE[T] = build_nc(T)
    nc = _NC_CACHE[T]
    in_maps = shard_inputs(x, W_attn, W_proj, T)
    res = run_bass_kernel_spmd(nc, in_maps, core_ids=list(range(8)), trace=trace)
    return gather_outputs(res.results, T), res


def kernel(x, W_attn, W_proj):
    out, _ = run(x, W_attn, W_proj, T=2048)
    return out


# revision 31
# speedup vs baseline: 1.2952x; 1.2952x over previous
"""Causal self-attention (B=4, T=2048, C=1024, H=16) on 8 TRN2 NeuronCores.

Sharding: core c = (b, hg) with b = c//2 batch index, hg = c%2 head-group
(8 heads each).  Each core computes its batch element's attention for its 8
heads plus the partial c_proj (W_proj column-shard); the host sums the two
head-group partials per batch element.

v3 (vs v2):
  - inputs (xT, WqkT, WvT) shipped in bf16: halves input DMA; stage-1
    matmuls run bf16 (same 1 cyc/row as f32r on PE).
  - S/O interleaved emission per (tqb, pc): the stage-3 matmuls for s-tile
    st-1 are emitted between the stage-2 matmuls of st, so the PE never
    head-of-line blocks on psum-bank recycling or on exp.
  - boundary s-tiles: stage-2 matmuls compute only the causally valid
    columns; exp is one merged instruction covering both heads (strided
    AP); the dead-strip memset and the triangle mask are one instruction
    each covering both heads.
  - normalize: one s2 psum tile holds both heads' reciprocal broadcasts
    (two 512-col matmuls); the yT muls read psO and psR directly (two
    PSUM operands); both heads' ytb shifted to partitions 64:127 with one
    DMA per tqb.

`reps` > 1 repeats the whole body inside one NEFF (for wall-clock timing by
differencing, since per-dispatch overhead through axon is ~70-90 ms).
"""
import numpy as np
import ml_dtypes

import concourse.bacc as bacc
import concourse.mybir as mybir
import concourse.tile as tile
from concourse.bass import broadcast_tensor_aps
from concourse.bass_utils import run_bass_kernel_spmd

F32 = mybir.dt.float32
F32R = mybir.dt.float32r
BF16 = mybir.dt.bfloat16

B, C, NH, HD = 4, 1024, 16, 64
HPC = 8              # heads per core
JV = HPC * HD        # 512: v-feature cols per core
KC = C // 128        # 8 contraction chunks
SCALE = 1.0 / 8.0    # 1/sqrt(HD)

# Schraudolph fast-exp in bf16 via DVE (offloads the ACT engine, which
# paces the attention loop on HW): bf16 bitpattern of exp(s/8) is
# approximately round(s * (128*log2(e)/8) + SEXP_C2); computed as one
# tensor_scalar (mult+add) with an int16-bitcast output AP.  Max rel
# error ~3.3% on individual weights; cancels to ~1e-3 after softmax
# normalization (numerator and denominator use the same approx values).
SEXP_C1 = 184.6617 / 8.0
SEXP_C2 = 16250.75


def emit_body(nc, tc, dram, T):
    TT = T // 128
    TQB = T // 512
    xT, wqkT, wvT, wpTb, tri, yout = (
        dram["xT"], dram["wqkT"], dram["wvT"], dram["wpTb"],
        dram["tri"], dram["yout"])

    with tc.tile_pool(name="persist", bufs=1) as pers:
        qkT_sb = pers.tile([128, 8, T], F32R)          # [j-part, jc, t]
        vext_sb = pers.tile([128, TT, HPC, 65], BF16)  # [s-part, st, h, d|1]
        tri2_sb = pers.tile([128, 2, 128], BF16)       # tri[p,c]=1 iff c>=p,
        selt = pers.tile([65, 64], F32R)               # ones row at p=64
        nc.sync.dma_start(tri2_sb[:, 0, :], tri[:])    # duplicated per head
        nc.sync.dma_start(tri2_sb[:, 1, :], tri[:])
        nc.sync.dma_start(selt[64:65, :], dram["ones64"][:])

        with tc.tile_pool(name="mmx", bufs=2, space="PSUM") as ps512:
            # ---------------- stage 1 ----------------
            with tc.tile_pool(name="stage1", bufs=1) as s1p:
                xT_sb = s1p.tile([128, KC, T], BF16)
                wqk_sb = s1p.tile([128, KC, 1024], BF16)
                wv_sb = s1p.tile([128, KC, JV], BF16)
                xT3 = xT.rearrange("(kc p) t -> p kc t", p=128)
                wqk3 = wqkT.rearrange("(kc p) j -> p kc j", p=128)
                wv3 = wvT.rearrange("(kc p) j -> p kc j", p=128)
                # DMA priority order: the first qk psum chains need wqk[kc]
                # plus the nb=0 slice of xT[kc]; ship those first so the PE
                # starts ~2 us in, then stream the rest of xT, then wv.
                for kc in range(KC):
                    nc.sync.dma_start(wqk_sb[:, kc, :], wqk3[:, kc, :])
                    nc.sync.dma_start(xT_sb[:, kc, 0:512], xT3[:, kc, 0:512])
                for kc in range(KC):
                    nc.sync.dma_start(xT_sb[:, kc, 512:T], xT3[:, kc, 512:T])
                for kc in range(KC):
                    nc.sync.dma_start(wv_sb[:, kc, :], wv3[:, kc, :])

                # qkT = WqkT.T-contract(xT): out chunk jc over t blocks
                for nb in range(T // 512):
                    for jc in range(8):
                        ps = ps512.tile([128, 512], F32, tag="ps512")
                        for kc in range(KC):
                            nc.tensor.matmul(
                                ps[:],
                                wqk_sb[:, kc, jc * 128:(jc + 1) * 128],
                                xT_sb[:, kc, nb * 512:(nb + 1) * 512],
                                start=(kc == 0), stop=(kc == KC - 1))
                        nc.vector.tensor_copy(
                            qkT_sb[:, jc, nb * 512:(nb + 1) * 512], ps[:])
                # V = xT.T-contract(WvT): out t-chunk tt, 512 v-cols
                for tt in range(TT):
                    ps = ps512.tile([128, 512], F32, tag="ps512")
                    for kc in range(KC):
                        nc.tensor.matmul(
                            ps[:],
                            xT_sb[:, kc, tt * 128:(tt + 1) * 128],
                            wv_sb[:, kc, :],
                            start=(kc == 0), stop=(kc == KC - 1))
                    nc.vector.tensor_copy(
                        vext_sb[:, tt, :, 0:64],
                        ps[:].rearrange("p (h d) -> p h d", h=HPC))
                    nc.vector.memset(vext_sb[:, tt, :, 64:65], 1.0)

            # ---------------- attention + proj ----------------
            with tc.tile_pool(name="s2ps", bufs=2, space="PSUM") as s2ps, \
                 tc.tile_pool(name="ps3p", bufs=2, space="PSUM") as ps3p, \
                 tc.tile_pool(name="wp", bufs=1) as wpp, \
                 tc.tile_pool(name="pexp", bufs=2) as ppool, \
                 tc.tile_pool(name="ytpool", bufs=2) as ytpool, \
                 tc.tile_pool(name="ybpool", bufs=2) as ybpool, \
                 tc.tile_pool(name="rbpool", bufs=4) as rbpool, \
                 tc.tile_pool(name="outp", bufs=3) as outp:
                wp_sb = wpp.tile([128, 4, C], BF16)
                wp3 = wpTb.rearrange("(jc p) co -> p jc co", p=128)
                for jc in range(4):
                    nc.sync.dma_start(wp_sb[:, jc, :], wp3[:, jc, :])

                def emit_s4_chunk(yT_s4, tqb_s4, k):
                    """Stage-4 chunk k (0..7) of tq block tqb_s4: one psum
                    group = 128 t-rows x 512 out-cols."""
                    sub, nb2 = k // 2, k % 2
                    ps4 = ps512.tile([128, 512], F32, tag="ps512")
                    for jc in range(4):
                        nc.tensor.matmul(
                            ps4[:],
                            yT_s4[:, jc, sub * 128:(sub + 1) * 128],
                            wp_sb[:, jc, nb2 * 512:(nb2 + 1) * 512],
                            start=(jc == 0), stop=(jc == 3))
                    ot = outp.tile([128, 512], F32, tag="ot")
                    nc.vector.tensor_copy(ot[:], ps4[:])
                    t0 = (tqb_s4 * 4 + sub) * 128
                    nc.sync.dma_start(
                        yout[t0:t0 + 128, nb2 * 512:(nb2 + 1) * 512],
                        ot[:])

                yT_prev = None
                for tqb in range(TQB):
                    nst = 4 * (tqb + 1)     # causal: s-tiles 0..nst-1
                    yT_t = ytpool.tile([128, 4, 512], BF16, tag="yt")
                    for pc in range(4):
                        pab = ppool.tile([128, TT, 1024], BF16, tag="pab")
                        qs = 2 * pc         # chunk with [Qa|Qb]
                        ks = 2 * pc + 1     # chunk with [Ka|Kb]
                        tqs = slice(tqb * 512, (tqb + 1) * 512)
                        psOa = ps3p.tile([128, 512], F32, tag="s3")
                        psOb = ps3p.tile([128, 512], F32, tag="s3")
                        psOs = (psOa, psOb)

                        def emit_o(st, start, stop):
                            for hoff in (0, 1):
                                nc.tensor.matmul(
                                    psOs[hoff][0:65, :],
                                    vext_sb[:, st, 2 * pc + hoff, :],
                                    pab[:, st, hoff * 512:(hoff + 1) * 512],
                                    start=start, stop=stop)

                        for st in range(nst):
                            ss = slice(st * 128, (st + 1) * 128)
                            psAB = s2ps.tile([128, 1024], F32, tag="s2")
                            q = st - 4 * tqb
                            c0 = max(q, 0) * 128   # first valid col
                            tqv = slice(tqb * 512 + c0, (tqb + 1) * 512)
                            nc.tensor.matmul(
                                psAB[:, c0:512], qkT_sb[0:64, ks, ss],
                                qkT_sb[0:64, qs, tqv],
                                start=True, stop=True, tile_position=(0, 0))
                            nc.tensor.matmul(
                                psAB[:, 512 + c0:1024], qkT_sb[64:128, ks, ss],
                                qkT_sb[64:128, qs, tqv],
                                start=True, stop=True, tile_position=(64, 0))
                            pab2 = pab[:, st, :].rearrange(
                                "p (h c) -> p h c", h=2)
                            psAB2 = psAB.rearrange("p (h c) -> p h c", h=2)
                            if q < 0:       # fully-valid s-tile
                                if st % 2 == 1:   # DVE fast-exp offload
                                    with nc.allow_low_precision(
                                            reason="schraudolph bf16 exp"):
                                        nc.vector.tensor_scalar(
                                            pab[:, st, :].bitcast(
                                                mybir.dt.int16),
                                            psAB[:], SEXP_C1, SEXP_C2,
                                            mybir.AluOpType.mult,
                                            mybir.AluOpType.add)
                                else:
                                    nc.scalar.activation(
                                        pab[:, st, :], psAB[:],
                                        mybir.ActivationFunctionType.Exp,
                                        scale=SCALE)
                            else:           # boundary s-tile: causal edge
                                if q > 0:
                                    nc.gpsimd.memset(pab2[:, :, 0:c0], 0.0)
                                nc.scalar.activation(
                                    pab2[:, :, c0:512], psAB2[:, :, c0:512],
                                    mybir.ActivationFunctionType.Exp,
                                    scale=SCALE)
                                mpa = pab2[:, :, c0:c0 + 128]
                                nc.gpsimd.tensor_mul(mpa, mpa, tri2_sb[:])
                            if st >= 2:
                                emit_o(st - 2, start=(st == 2), stop=False)
                        emit_o(nst - 2, start=(nst == 2), stop=False)
                        emit_o(nst - 1, start=False, stop=True)

                        # normalize: yT_h = OT_h * (1/denom_h); reciprocal of
                        # the denom row stays on partition 64 (lane-aligned),
                        # then a K=1 ones-row matmul broadcasts it across 64
                        # output partitions.
                        rcp2 = rbpool.tile([65, 2, 512], F32R, tag="rcp")
                        with nc.allow_low_precision(
                                reason="f32r is bit-identical to f32; "
                                       "needed for 1-cyc/row PE broadcast"):
                            nc.vector.reciprocal(
                                rcp2[64:65, 0, :], psOa[64:65, :])
                            nc.vector.reciprocal(
                                rcp2[64:65, 1, :], psOb[64:65, :])
                        psRa = ps512.tile([128, 512], F32, tag="ps512")
                        psRb = ps512.tile([128, 512], F32, tag="ps512")
                        nc.tensor.matmul(
                            psRa[0:64, :], selt[64:65, :],
                            rcp2[64:65, 0, :], start=True, stop=True)
                        nc.tensor.matmul(
                            psRb[0:64, :], selt[64:65, :],
                            rcp2[64:65, 1, :], start=True, stop=True)
                        rba = rbpool.tile([64, 512], F32, tag="rba")
                        nc.vector.tensor_copy(rba[:], psRa[0:64, :])
                        rbb = rbpool.tile([64, 512], F32, tag="rbb")
                        nc.vector.tensor_copy(rbb[:], psRb[0:64, :])
                        ytb = ybpool.tile([64, 512], BF16, tag="ytb")
                        with nc.allow_low_precision(
                                reason="attn output feeds bf16 c_proj"):
                            nc.vector.tensor_mul(
                                yT_t[0:64, pc, :], psOa[0:64, :], rba[:])
                            nc.vector.tensor_mul(
                                ytb[:], psOb[0:64, :], rbb[:])
                        nc.sync.dma_start(yT_t[64:128, pc, :], ytb[:])
                        # stage 4 of the previous tq block: PE filler that
                        # has no ACT/DVE dependency, absorbing exp-lag and
                        # normalize-latency stalls.
                        if yT_prev is not None:
                            emit_s4_chunk(yT_prev, tqb - 1, 2 * pc)
                            emit_s4_chunk(yT_prev, tqb - 1, 2 * pc + 1)
                    yT_prev = yT_t
                # stage 4 of the last tq block
                for k in range(8):
                    emit_s4_chunk(yT_prev, TQB - 1, k)


def build_nc(T=2048, reps=1, loop_reps=None):
    """reps: python-unrolled body repeats.  loop_reps: wrap ONE body in a
    tc.For_i hardware loop of that trip count (NEFF size independent of
    trip count — for clean timing differencing)."""
    nc = bacc.Bacc()
    dram = dict(
        xT=nc.dram_tensor("xT", [C, T], BF16, kind="ExternalInput"),
        wqkT=nc.dram_tensor("wqkT", [C, 1024], BF16, kind="ExternalInput"),
        wvT=nc.dram_tensor("wvT", [C, JV], BF16, kind="ExternalInput"),
        wpTb=nc.dram_tensor("wpTb", [JV, C], BF16, kind="ExternalInput"),
        tri=nc.dram_tensor("tri", [128, 128], BF16, kind="ExternalInput"),
        ones64=nc.dram_tensor("ones64", [1, 64], F32R, kind="ExternalInput"),
        yout=nc.dram_tensor("yout", [T, C], F32, kind="ExternalOutput"),
    )
    with tile.TileContext(nc) as tc:
        if loop_reps is not None:
            with tc.For_i(0, loop_reps):
                emit_body(nc, tc, dram, T)
        else:
            for _ in range(reps):
                emit_body(nc, tc, dram, T)
    nc.compile()
    return nc


def shard_inputs(x, W_attn, W_proj, T):
    """Full inputs -> list of 8 per-core in_maps."""
    x = np.asarray(x, dtype=np.float32)
    W_attn = np.asarray(W_attn, dtype=np.float32)
    W_proj = np.asarray(W_proj, dtype=np.float32)

    p = np.arange(128)[:, None]
    c = np.arange(128)[None, :]
    tri = (c >= p).astype(ml_dtypes.bfloat16)

    in_maps = []
    for core in range(8):
        b, hg = core // 2, core % 2
        heads = [hg * HPC + i for i in range(HPC)]
        cols = []
        for pc in range(4):
            ha, hb = heads[2 * pc], heads[2 * pc + 1]
            cols += list(range(ha * 192, ha * 192 + 64))        # Q_a
            cols += list(range(hb * 192, hb * 192 + 64))        # Q_b
            cols += list(range(ha * 192 + 64, ha * 192 + 128))  # K_a
            cols += list(range(hb * 192 + 64, hb * 192 + 128))  # K_b
        vrows = [h * 192 + 128 + d for h in heads for d in range(64)]
        in_maps.append(dict(
            xT=np.ascontiguousarray(x[b, :T].T).astype(ml_dtypes.bfloat16),
            wqkT=np.ascontiguousarray(
                W_attn[cols].T).astype(ml_dtypes.bfloat16),
            wvT=np.ascontiguousarray(
                W_attn[vrows].T).astype(ml_dtypes.bfloat16),
            tri=tri,
            ones64=np.ones((1, 64), dtype=np.float32),
            wpTb=np.ascontiguousarray(
                W_proj[:, hg * JV:(hg + 1) * JV].T).astype(ml_dtypes.bfloat16),
        ))
    return in_maps


def gather_outputs(results, T):
    out = np.empty((B, T, C), dtype=np.float32)
    for b in range(B):
        out[b] = results[2 * b]["yout"] + results[2 * b + 1]["yout"]
    return out


_NC_CACHE = {}


def run(x, W_attn, W_proj, T=2048, trace=False):
    if T not in _NC_CACHE:
        _NC_CACHE[T] = build_nc(T)
    nc = _NC_CACHE[T]
    in_maps = shard_inputs(x, W_attn, W_proj, T)
    res = run_bass_kernel_spmd(nc, in_maps, core_ids=list(range(8)), trace=trace)
    return gather_outputs(res.results, T), res


def kernel(x, W_attn, W_proj):
    out, _ = run(x, W_attn, W_proj, T=2048)
    return out
